# revision 36
# baseline (speedup 1.0000x reference)
"""GroupQueryAttention on 8 TRN2 NeuronCores.  (v3 snapshot: 445,905 ns)

Strategy: tensor-parallel over heads. H=32 query heads, KV=8 kv heads,
group size G=4 -> each core owns exactly 1 kv head and its 4 query heads.
Per core:
  - QKV projections from a replicated (pre-transposed, channels-major) input
  - RoPE on Q/K (rotate-half, done on DVE across partition halves)
  - attention with scores computed TRANSPOSED ([keys, q] layout) so the
    exp(scores) tiles feed the V-matmul directly as the moving operand
    (no P-transpose needed); softmax normalization is deferred: O = V.E,
    then ctx = O * (1/colsum(E)) broadcast via a rank-1 matmul
  - partial output ctx @ Wo_shard  (row-shard of Wo)
Host sums the 8 partial outputs (the "all-reduce" of the row-parallel Wo).

v3 perf structure:
  - score/exp tiles in key-tile PAIRS ([128, 2, 512] PSUM; wide ACT exp)
  - PE software pipeline: scores for pair p+1 issue before PV of pair p
  - reciprocal_approx_fast for the softmax denominator
  - output projection groups accumulated in the wide ps_sc pool; each
    quarter's out-proj groups are interleaved into the NEXT quarter's
    attention as PE filler work (the attention phase is otherwise
    exp/ACT-bound and the PE would idle)
  - x input single-buffered (its reload hides under attention+filler),
    qt reduced to a per-quarter ring, Wo resident from the start
  - PSUM->SBUF drains split across ACT and DVE; output DMA'd per group

Causal mask: the attention_mask input is verified on host to be the
upper-triangular causal mask; the device program exploits causality by
skipping fully-masked key-tiles and applying 0/1 mask tiles on the 4
diagonal-crossing key-tiles of each query chunk. If the mask is ever not
causal, a numpy fallback computes the exact reference on host.

Compute dtype: bf16 on the PE (f32 PSUM accumulation), f32 softmax
bookkeeping. Output partials returned as bf16, summed on host in f32.
"""

import sys

sys.path.insert(0, "/opt/trn_rl_repo")

from contextlib import ExitStack

import numpy as np
import ml_dtypes

import concourse.bass as bass
import concourse.bacc as bacc
import concourse.tile as tile
from concourse import mybir
from concourse.bass_utils import run_bass_kernel_spmd

BF16 = ml_dtypes.bfloat16

S = 2048          # sequence length
DIN = 4096        # model dim
H, KV, DH = 32, 8, 128
G = H // KV       # 4 query heads per kv head
NCORES = 8
HPC = H // NCORES     # 4 query heads per core
DPC = HPC * DH        # 512 = per-core q-projection width

NQ = 4            # s-quarters (chunks of 512 queries)
QC = S // NQ      # 512
KT = 128          # key tile (partition dim of transposed scores)
NKT = S // KT     # 16 key tiles
NK = DIN // 128   # 32 contraction tiles for projections
SCALE = 1.0 / float(np.sqrt(DH))
EXP_BIAS = -10.0  # constant shift inside exp; cancels in normalization
OCW = 1024        # out-projection moving width
NOC = DIN // OCW  # 4 wide column groups


def build_nc():
    """Build the per-core Bass program (same program on all 8 cores; the
    per-core weight shards arrive via in_maps)."""
    nc = bacc.Bacc()
    dt = mybir.dt

    # ---- DRAM parameters (host-prepared layouts; all DMA-contiguous) ----
    # x[p, sq, k, sc] = x_orig[512*sq + sc, 128*k + p]   (channels-major)
    x = nc.declare_dram_parameter("x", [128, NQ, NK, QC], dt.bfloat16, isOutput=False)
    # wq[p, k, m, d] = Wq_shard[128*k + p, 128*m + d]
    wq = nc.declare_dram_parameter("wq", [128, NK, HPC, DH], dt.bfloat16, isOutput=False)
    # wk[p, k, d] = Wk_shard[128*k + p, d]
    wk = nc.declare_dram_parameter("wk", [128, NK, DH], dt.bfloat16, isOutput=False)
    wv = nc.declare_dram_parameter("wv", [128, NK, DH], dt.bfloat16, isOutput=False)
    # wo[p, h, n] = Wo_shard[128*h + p, n]
    wo = nc.declare_dram_parameter("wo", [128, HPC, DIN], dt.bfloat16, isOutput=False)
    # cosT[d, s] = cos[s, d]; sinm[d, s] = -sin[s, d] for d<64 else +sin[s, d]
    cosT = nc.declare_dram_parameter("cosT", [DH, S], dt.bfloat16, isOutput=False)
    sinm = nc.declare_dram_parameter("sinm", [DH, S], dt.bfloat16, isOutput=False)
    # m01[p, r, f] = 0.0 where 128*r + p > f else 1.0  (diagonal-tile masks)
    m01 = nc.declare_dram_parameter("m01", [128, 4, QC], dt.bfloat16, isOutput=False)
    ident = nc.declare_dram_parameter("ident", [128, 128], dt.bfloat16, isOutput=False)
    ones_col = nc.declare_dram_parameter("ones_col", [128, 1], dt.bfloat16, isOutput=False)
    ones_row = nc.declare_dram_parameter("ones_row", [1, 128], dt.float32, isOutput=False)
    out = nc.declare_dram_parameter("out", [S, DIN], dt.bfloat16, isOutput=True)

    with tile.TileContext(nc) as tc, ExitStack() as ctx:
        singles = ctx.enter_context(tc.tile_pool(name="singles", bufs=1))
        xpool = ctx.enter_context(tc.tile_pool(name="xpool", bufs=1))
        qpool = ctx.enter_context(tc.tile_pool(name="qpool", bufs=2))
        qkv = ctx.enter_context(tc.tile_pool(name="qkv", bufs=1))
        epool = ctx.enter_context(tc.tile_pool(name="epool", bufs=3))
        spool = ctx.enter_context(tc.tile_pool(name="spool", bufs=2))
        tpool = ctx.enter_context(tc.tile_pool(name="tpool", bufs=2))
        obuf = ctx.enter_context(tc.tile_pool(name="obuf", bufs=4))
        ps_acc = ctx.enter_context(tc.tile_pool(name="ps_acc", bufs=2, space="PSUM"))
        ps_sc = ctx.enter_context(tc.tile_pool(name="ps_sc", bufs=2, space="PSUM"))
        ps_sm = ctx.enter_context(tc.tile_pool(name="ps_sm", bufs=2, space="PSUM"))

        # ---- HAM warm-up ----
        # The PE sits idle ~10us at kernel start waiting for the first
        # weight/input DMAs, so the HAM clock gate keeps it at 1.2 GHz for
        # the first ~3.4us of real matmuls.  16 dummy matmuls on zeroed
        # scratch (no DMA dependency) fill that idle window and un-throttle
        # the clock before the real projections begin.
        warm = singles.tile([128, 640], dt.bfloat16, tag="warm")
        nc.vector.memset(warm, 0.0)
        for wg in range(2):
            pswarm = ps_sm.tile([128, 512], dt.float32, tag="sm", name="pswarm")
            for j in range(8):
                nc.tensor.matmul(pswarm, lhsT=warm[:, 0:128], rhs=warm[:, 128:640],
                                 start=(j == 0), stop=(j == 7))

        # ---- constants / weights resident in SBUF ----
        # DMA emission order matters: the K-projection consumes w_k and the
        # first x sub-tiles, so those go first; everything else follows in
        # consumption order to keep the PE from stalling at kernel start.
        # The K projection consumes (w_k chunk, x chunk) pairs sequentially;
        # the chunked interleave lets its first matmuls start ~3us in.  Wq
        # head 0 is prioritized over cos/sin (the Q0 projection's LDWEIGHTS
        # is the next PE stall point; RoPE-K on DVE can wait a little).
        w_k = singles.tile([128, NK, DH], dt.bfloat16, tag="wk")
        x_t0 = xpool.tile([128, NK, QC], dt.bfloat16, tag="xq", name="x_t0")
        for g in range(4):
            nc.sync.dma_start(out=w_k[:, g * 8:(g + 1) * 8],
                              in_=wk[:, g * 8:(g + 1) * 8])
            nc.sync.dma_start(out=x_t0[:, g * 8:(g + 1) * 8],
                              in_=x[:, 0, g * 8:(g + 1) * 8])

        w_q = singles.tile([128, NK, HPC, DH], dt.bfloat16, tag="wq")
        nc.sync.dma_start(out=w_q[:, :, 0], in_=wq[:, :, 0])

        c_cos = singles.tile([DH, S], dt.bfloat16, tag="cos")
        nc.sync.dma_start(out=c_cos, in_=cosT[:])
        c_sin = singles.tile([DH, S], dt.bfloat16, tag="sin")
        nc.sync.dma_start(out=c_sin, in_=sinm[:])

        for h in range(1, HPC):
            for g2 in range(2):
                nc.sync.dma_start(out=w_q[:, g2 * 16:(g2 + 1) * 16, h],
                                  in_=wq[:, g2 * 16:(g2 + 1) * 16, h])
        w_v = singles.tile([128, NK, DH], dt.bfloat16, tag="wv")
        nc.sync.dma_start(out=w_v, in_=wv[:])

        c_m01 = singles.tile([128, 4, QC], dt.bfloat16, tag="m01")
        nc.sync.dma_start(out=c_m01, in_=m01[:])
        c_id = singles.tile([128, 128], dt.bfloat16, tag="ident")
        nc.sync.dma_start(out=c_id, in_=ident[:])
        c_oc = singles.tile([128, 1], dt.bfloat16, tag="ones_col")
        nc.sync.dma_start(out=c_oc, in_=ones_col[:])
        c_or = singles.tile([1, 128], dt.float32, tag="ones_row")
        nc.sync.dma_start(out=c_or, in_=ones_row[:])
        c_bias = singles.tile([128, 1], dt.float32, tag="ebias")
        nc.vector.memset(c_bias, EXP_BIAS)

        # Wo resident from the start (first consumed during quarter-1
        # attention by the interleaved out-projection of quarter 0).
        w_o = singles.tile([128, HPC, DIN], dt.bfloat16, tag="wo")
        nc.sync.dma_start(out=w_o, in_=wo[:])

        # ---- long-lived activations ----
        kt = qkv.tile([DH, S], dt.bfloat16, tag="kt")
        vn = qkv.tile([128, NKT, DH], dt.bfloat16, tag="vn")     # V natural [j, d] tiles
        ctxT = [qkv.tile([DH, S], dt.bfloat16, tag=f"ctx{h}", name=f"ctx{h}") for h in range(HPC)]

        def rope_from_psum(ps, dst_slice, s0):
            """dst = ps*cos + rot_half(ps)*sinm over s-columns [s0, s0+QC)."""
            t1 = tpool.tile([DH, QC], dt.float32, tag="t1", name="t1")
            nc.vector.tensor_mul(t1, ps, c_cos[:, s0:s0 + QC])
            t2 = tpool.tile([DH, QC], dt.float32, tag="t2", name="t2")
            nc.vector.tensor_mul(t2[0:64, :], ps[64:128, :], c_sin[0:64, s0:s0 + QC])
            nc.vector.tensor_mul(t2[64:128, :], ps[0:64, :], c_sin[64:128, s0:s0 + QC])
            nc.vector.tensor_add(dst_slice, t1, t2)

        def emit_oproj_group(sq, st, ocw, alt):
            """One 1024-wide out-projection group: out rows of st-tile,
            columns [ocw*1024, (ocw+1)*1024). 8 matmuls + drain + DMA."""
            pso = ps_sc.tile([128, 2, QC], dt.float32, tag="sc", name="pso")
            for i in range(2):
                oc0 = ocw * OCW + i * QC
                for h in range(HPC):
                    nc.tensor.matmul(pso[:, i],
                                     lhsT=ctxT[h][:, st * 128:(st + 1) * 128],
                                     rhs=w_o[:, h, oc0:oc0 + QC],
                                     start=(h == 0), stop=(h == HPC - 1))
            if len(pending) == 0:
                # final group: split the drain in four so the end-of-kernel
                # copy+DMA tail is shorter
                for j4 in range(4):
                    sth = obuf.tile([128, QC // 2], dt.bfloat16, tag="sth", name="sth")
                    if j4 % 2 == 0:
                        nc.scalar.copy(sth, pso[:, j4 // 2, (j4 % 2) * 256:(j4 % 2) * 256 + 256])
                    else:
                        nc.vector.tensor_copy(sth, pso[:, j4 // 2, (j4 % 2) * 256:(j4 % 2) * 256 + 256])
                    nc.sync.dma_start(
                        out=out[st * 128:(st + 1) * 128,
                                ocw * OCW + j4 * 256:ocw * OCW + (j4 + 1) * 256],
                        in_=sth)
                return
            stage = obuf.tile([128, OCW], dt.bfloat16, tag="stage", name="stage")
            if alt % 2 == 0:
                nc.scalar.copy(stage, pso)
            else:
                nc.vector.tensor_copy(stage, pso)
            nc.sync.dma_start(
                out=out[st * 128:(st + 1) * 128, ocw * OCW:(ocw + 1) * OCW],
                in_=stage)

        pending = []   # deferred out-proj groups of the previous quarter
        alt_ctr = [0]

        def pop_fillers(k):
            for _ in range(min(k, len(pending))):
                sq_, st_, ocw_ = pending.pop(0)
                emit_oproj_group(sq_, st_, ocw_, alt_ctr[0])
                alt_ctr[0] += 1

        for sq in range(NQ):
            s0 = sq * QC
            if sq == 0:
                x_t = x_t0
            else:
                x_t = xpool.tile([128, NK, QC], dt.bfloat16, tag="xq", name="x_t")
                for g in range(4):
                    nc.sync.dma_start(out=x_t[:, g * 8:(g + 1) * 8],
                                      in_=x[:, sq, g * 8:(g + 1) * 8])

            # K projection + RoPE
            psk = ps_acc.tile([DH, QC], dt.float32, tag="acc", name="psk")
            for k in range(NK):
                nc.tensor.matmul(psk, lhsT=w_k[:, k], rhs=x_t[:, k],
                                 start=(k == 0), stop=(k == NK - 1))
            rope_from_psum(psk, kt[:, s0:s0 + QC], s0)

            # Q projections + RoPE (per-quarter ring; consumed by this
            # quarter's attention only)
            qt = []
            for h in range(HPC):
                psq = ps_acc.tile([DH, QC], dt.float32, tag="acc", name="psq")
                for k in range(NK):
                    nc.tensor.matmul(psq, lhsT=w_q[:, k, h], rhs=x_t[:, k],
                                     start=(k == 0), stop=(k == NK - 1))
                qth = qpool.tile([DH, QC], dt.bfloat16, tag=f"qt{h}", name=f"qt{h}")
                rope_from_psum(psq, qth, s0)
                qt.append(qth)

            # V projection (transposed layout), then PE-transpose to natural
            psv = ps_acc.tile([DH, QC], dt.float32, tag="acc", name="psv")
            for k in range(NK):
                nc.tensor.matmul(psv, lhsT=w_v[:, k], rhs=x_t[:, k],
                                 start=(k == 0), stop=(k == NK - 1))
            vtmp = tpool.tile([DH, QC], dt.bfloat16, tag="vtmp", name="vtmp", bufs=1)
            nc.scalar.copy(vtmp, psv)
            for i in range(QC // 128):
                pvt = ps_sm.tile([128, 128], dt.bfloat16, tag="sm", name="pvt")
                nc.tensor.transpose(pvt, vtmp[:, i * 128:(i + 1) * 128], c_id)
                nc.scalar.copy(vn[:, sq * 4 + i], pvt)

            # ---- attention for this quarter's queries (causal) ----
            # Key tiles in PAIRS: scores for a pair land in one [128, 2, QC]
            # PSUM tile (2 banks) so a single wide exp covers both; the PE
            # runs one pair ahead of the PV matmuls to hide exp latency, and
            # the previous quarter's out-proj groups fill the remaining PE
            # idle (the attention phase is ACT-bound on exp).
            npairs = 2 * (sq + 1)
            njt = 2 * npairs
            for h in range(HPC):
                po = ps_acc.tile([DH, QC], dt.float32, tag="acc", name="po")
                swide = spool.tile([128, 2, QC], dt.bfloat16, tag="sacc", name="swide")

                def emit_scores(pr):
                    psc = ps_sc.tile([128, 2, QC], dt.float32, tag="sc", name="psc")
                    for half in range(2):
                        jt = 2 * pr + half
                        nc.tensor.matmul(psc[:, half],
                                         lhsT=kt[:, jt * KT:(jt + 1) * KT],
                                         rhs=qt[h], start=True, stop=True)
                    e = epool.tile([128, 2, QC], dt.bfloat16, tag="e", name="e")
                    nc.scalar.activation(out=e, in_=psc,
                                         func=mybir.ActivationFunctionType.Exp,
                                         bias=c_bias, scale=SCALE)
                    r = pr - (npairs - 2)
                    if r >= 0:
                        nc.vector.tensor_mul(e, e, c_m01[:, 2 * r:2 * r + 2])
                    return e

                def emit_pv(pr, e):
                    for half in range(2):
                        jt = 2 * pr + half
                        nc.tensor.matmul(po, lhsT=vn[:, jt], rhs=e[:, half],
                                         start=(jt == 0), stop=(jt == njt - 1))

                e_prev = emit_scores(0)
                for pr in range(1, npairs):
                    e_cur = emit_scores(pr)
                    emit_pv(pr - 1, e_prev)
                    if pr == 1:
                        nc.vector.tensor_add(swide, e_prev, e_cur)
                    else:
                        nc.vector.tensor_add(swide, swide, e_cur)
                    e_prev = e_cur
                emit_pv(npairs - 1, e_prev)

                # fold pair slots; PE filler (previous quarter's out-proj)
                # runs while the DVE/ACT softmax bookkeeping drains
                sfin = spool.tile([128, QC], dt.bfloat16, tag="sfin", name="sfin")
                nc.vector.tensor_add(sfin, swide[:, 0], swide[:, 1])
                pop_fillers(4)

                # normalization: ctx = O * (1 / colsum(E)) broadcast over d
                pcs = ps_sm.tile([1, QC], dt.float32, tag="sm", name="pcs")
                nc.tensor.matmul(pcs, lhsT=c_oc, rhs=sfin, start=True, stop=True)
                rec = tpool.tile([1, QC], dt.float32, tag="rec", name="rec", bufs=1)
                nc.vector.reciprocal_approx_fast(rec, pcs)
                prb = ps_sm.tile([128, QC], dt.float32, tag="sm", name="prb")
                nc.tensor.matmul(prb, lhsT=c_or, rhs=rec, start=True, stop=True)
                rbs = tpool.tile([128, QC], dt.float32, tag="rbs", name="rbs")
                nc.vector.tensor_copy(rbs, prb)
                nc.vector.tensor_mul(ctxT[h][:, s0:s0 + QC], po, rbs)

            # queue this quarter's out-proj groups (needs all 4 heads' ctxT)
            for st in range(4 * sq, 4 * sq + 4):
                for ocw in range(NOC):
                    pending.append((sq, st, ocw))

        # flush the last quarter's out-projection
        pop_fillers(len(pending))
    nc.finalize()
    return nc


def make_in_maps(input_tensor, cos, sin, Wq, Wk, Wv, Wo):
    """Host-side sharding + layout preparation. Returns list of 8 dicts."""
    x2 = np.ascontiguousarray(input_tensor.reshape(S, DIN))
    # x_host[p, sq, k, sc] = x2[512*sq+sc, 128*k+p]
    xt = x2.T.astype(BF16)                      # [DIN, S]
    x_host = np.ascontiguousarray(
        xt.reshape(NK, 128, NQ, QC).transpose(1, 2, 0, 3))

    cosT = np.ascontiguousarray(cos.T.astype(BF16))
    sinm = sin.T.astype(np.float32).copy()
    sinm[0:64, :] *= -1.0
    sinm = np.ascontiguousarray(sinm.astype(BF16))

    p_idx = np.arange(128)[:, None, None]
    r_idx = np.arange(4)[None, :, None]
    f_idx = np.arange(QC)[None, None, :]
    m01 = ((128 * r_idx + p_idx) <= f_idx).astype(BF16)

    ident = np.eye(128, dtype=BF16)
    ones_col = np.ones((128, 1), dtype=BF16)
    ones_row = np.ones((1, 128), dtype=np.float32)

    common = dict(x=x_host, cosT=cosT, sinm=sinm, m01=m01, ident=ident,
                  ones_col=ones_col, ones_row=ones_row)

    in_maps = []
    for c in range(NCORES):
        wq_s = Wq[:, c * DPC:(c + 1) * DPC].astype(BF16)
        wq_host = np.ascontiguousarray(
            wq_s.reshape(NK, 128, HPC, DH).transpose(1, 0, 2, 3))
        wk_s = Wk[:, c * DH:(c + 1) * DH].astype(BF16)
        wk_host = np.ascontiguousarray(wk_s.reshape(NK, 128, DH).transpose(1, 0, 2))
        wv_s = Wv[:, c * DH:(c + 1) * DH].astype(BF16)
        wv_host = np.ascontiguousarray(wv_s.reshape(NK, 128, DH).transpose(1, 0, 2))
        wo_s = Wo[c * DPC:(c + 1) * DPC, :].astype(BF16)
        wo_host = np.ascontiguousarray(wo_s.reshape(HPC, 128, DIN).transpose(1, 0, 2))
        in_maps.append(dict(common, wq=wq_host, wk=wk_host, wv=wv_host, wo=wo_host))
    return in_maps


def _numpy_fallback(input_tensor, attention_mask, cos, sin, Wq, Wk, Wv, Wo):
    x = input_tensor.astype(np.float32)
    b, s, _ = x.shape
    q = (x @ Wq).reshape(b, s, H, DH).transpose(0, 2, 1, 3)
    k = (x @ Wk).reshape(b, s, KV, DH).transpose(0, 2, 1, 3)
    v = (x @ Wv).reshape(b, s, KV, DH).transpose(0, 2, 1, 3)

    def rope(t):
        t1, t2 = t[..., :64], t[..., 64:]
        rot = np.concatenate([-t2, t1], axis=-1)
        return t * cos[None, None] + rot * sin[None, None]

    q, k = rope(q), rope(k)
    k = np.repeat(k, G, axis=1)
    v = np.repeat(v, G, axis=1)
    sc = np.einsum('bhqd,bhkd->bhqk', q, k)
    sc = np.where(attention_mask, -np.inf, sc) / np.float32(np.sqrt(DH))
    sc = sc - sc.max(axis=-1, keepdims=True)
    w = np.exp(sc)
    w = w / w.sum(axis=-1, keepdims=True)
    ctx = np.einsum('bhqk,bhkd->bhqd', w, v)
    ctx = ctx.transpose(0, 2, 1, 3).reshape(b, s, H * DH)
    return (ctx @ Wo).astype(np.float32)


_NC_CACHE = {}


def kernel(input_tensor, attention_mask, cos, sin, Wq, Wk, Wv, Wo):
    mask = np.asarray(attention_mask).reshape(S, S)
    causal = np.array_equal(mask, np.triu(np.ones((S, S), bool), k=1))
    if not causal:
        return _numpy_fallback(np.asarray(input_tensor), np.asarray(attention_mask),
                               np.asarray(cos), np.asarray(sin),
                               np.asarray(Wq), np.asarray(Wk),
                               np.asarray(Wv), np.asarray(Wo))

    if "nc" not in _NC_CACHE:
        _NC_CACHE["nc"] = build_nc()
    nc = _NC_CACHE["nc"]

    in_maps = make_in_maps(np.asarray(input_tensor), np.asarray(cos),
                           np.asarray(sin), np.asarray(Wq), np.asarray(Wk),
                           np.asarray(Wv), np.asarray(Wo))
    res = run_bass_kernel_spmd(nc, in_maps, core_ids=list(range(NCORES)))
    acc = np.zeros((S, DIN), np.float32)
    for r in res.results:
        acc += np.asarray(r["out"], dtype=np.float32)
    return acc.reshape(1, S, DIN)


# revision 37
# speedup vs baseline: 1.0067x; 1.0067x over previous
"""GroupQueryAttention on 8 TRN2 NeuronCores.  (v3 snapshot: 445,905 ns)

Strategy: tensor-parallel over heads. H=32 query heads, KV=8 kv heads,
group size G=4 -> each core owns exactly 1 kv head and its 4 query heads.
Per core:
  - QKV projections from a replicated (pre-transposed, channels-major) input
  - RoPE on Q/K (rotate-half, done on DVE across partition halves)
  - attention with scores computed TRANSPOSED ([keys, q] layout) so the
    exp(scores) tiles feed the V-matmul directly as the moving operand
    (no P-transpose needed); softmax normalization is deferred: O = V.E,
    then ctx = O * (1/colsum(E)) broadcast via a rank-1 matmul
  - partial output ctx @ Wo_shard  (row-shard of Wo)
Host sums the 8 partial outputs (the "all-reduce" of the row-parallel Wo).

v3 perf structure:
  - score/exp tiles in key-tile PAIRS ([128, 2, 512] PSUM; wide ACT exp)
  - PE software pipeline: scores for pair p+1 issue before PV of pair p
  - reciprocal_approx_fast for the softmax denominator
  - output projection groups accumulated in the wide ps_sc pool; each
    quarter's out-proj groups are interleaved into the NEXT quarter's
    attention as PE filler work (the attention phase is otherwise
    exp/ACT-bound and the PE would idle)
  - x input single-buffered (its reload hides under attention+filler),
    qt reduced to a per-quarter ring, Wo resident from the start
  - PSUM->SBUF drains split across ACT and DVE; output DMA'd per group

Causal mask: the attention_mask input is verified on host to be the
upper-triangular causal mask; the device program exploits causality by
skipping fully-masked key-tiles and applying 0/1 mask tiles on the 4
diagonal-crossing key-tiles of each query chunk. If the mask is ever not
causal, a numpy fallback computes the exact reference on host.

Compute dtype: bf16 on the PE (f32 PSUM accumulation), f32 softmax
bookkeeping. Output partials returned as bf16, summed on host in f32.
"""

import sys

sys.path.insert(0, "/opt/trn_rl_repo")

from contextlib import ExitStack

import numpy as np
import ml_dtypes

import concourse.bass as bass
import concourse.bacc as bacc
import concourse.tile as tile
from concourse import mybir
from concourse.bass_utils import run_bass_kernel_spmd

BF16 = ml_dtypes.bfloat16

S = 2048          # sequence length
DIN = 4096        # model dim
H, KV, DH = 32, 8, 128
G = H // KV       # 4 query heads per kv head
NCORES = 8
HPC = H // NCORES     # 4 query heads per core
DPC = HPC * DH        # 512 = per-core q-projection width

NQ = 4            # s-quarters (chunks of 512 queries)
QC = S // NQ      # 512
KT = 128          # key tile (partition dim of transposed scores)
NKT = S // KT     # 16 key tiles
NK = DIN // 128   # 32 contraction tiles for projections
SCALE = 1.0 / float(np.sqrt(DH))
EXP_BIAS = -10.0  # constant shift inside exp; cancels in normalization
OCW = 1024        # out-projection moving width
NOC = DIN // OCW  # 4 wide column groups


def build_nc():
    """Build the per-core Bass program (same program on all 8 cores; the
    per-core weight shards arrive via in_maps)."""
    nc = bacc.Bacc()
    dt = mybir.dt

    # ---- DRAM parameters (host-prepared layouts; all DMA-contiguous) ----
    # x[p, sq, k, sc] = x_orig[512*sq + sc, 128*k + p]   (channels-major)
    x = nc.declare_dram_parameter("x", [128, NQ, NK, QC], dt.bfloat16, isOutput=False)
    # wq[p, k, m, d] = Wq_shard[128*k + p, 128*m + d]
    wq = nc.declare_dram_parameter("wq", [128, NK, HPC, DH], dt.bfloat16, isOutput=False)
    # wk[p, k, d] = Wk_shard[128*k + p, d]
    wk = nc.declare_dram_parameter("wk", [128, NK, DH], dt.bfloat16, isOutput=False)
    wv = nc.declare_dram_parameter("wv", [128, NK, DH], dt.bfloat16, isOutput=False)
    # wo[p, h, n] = Wo_shard[128*h + p, n]
    wo = nc.declare_dram_parameter("wo", [128, HPC, DIN], dt.bfloat16, isOutput=False)
    # cosT[d, s] = cos[s, d]; sinm[d, s] = -sin[s, d] for d<64 else +sin[s, d]
    cosT = nc.declare_dram_parameter("cosT", [DH, S], dt.bfloat16, isOutput=False)
    sinm = nc.declare_dram_parameter("sinm", [DH, S], dt.bfloat16, isOutput=False)
    # m01[p, r, f] = 0.0 where 128*r + p > f else 1.0  (diagonal-tile masks)
    m01 = nc.declare_dram_parameter("m01", [128, 4, QC], dt.bfloat16, isOutput=False)
    ident = nc.declare_dram_parameter("ident", [128, 128], dt.bfloat16, isOutput=False)
    ones_col = nc.declare_dram_parameter("ones_col", [128, 1], dt.bfloat16, isOutput=False)
    ones_row = nc.declare_dram_parameter("ones_row", [1, 128], dt.float32, isOutput=False)
    out = nc.declare_dram_parameter("out", [S, DIN], dt.bfloat16, isOutput=True)

    with tile.TileContext(nc) as tc, ExitStack() as ctx:
        singles = ctx.enter_context(tc.tile_pool(name="singles", bufs=1))
        xpool = ctx.enter_context(tc.tile_pool(name="xpool", bufs=1))
        qpool = ctx.enter_context(tc.tile_pool(name="qpool", bufs=2))
        qkv = ctx.enter_context(tc.tile_pool(name="qkv", bufs=1))
        epool = ctx.enter_context(tc.tile_pool(name="epool", bufs=3))
        spool = ctx.enter_context(tc.tile_pool(name="spool", bufs=2))
        tpool = ctx.enter_context(tc.tile_pool(name="tpool", bufs=2))
        obuf = ctx.enter_context(tc.tile_pool(name="obuf", bufs=4))
        ps_acc = ctx.enter_context(tc.tile_pool(name="ps_acc", bufs=2, space="PSUM"))
        ps_sc = ctx.enter_context(tc.tile_pool(name="ps_sc", bufs=2, space="PSUM"))
        ps_sm = ctx.enter_context(tc.tile_pool(name="ps_sm", bufs=2, space="PSUM"))

        # ---- constants / weights resident in SBUF ----
        # DMA emission order matters: the K-projection consumes w_k and the
        # first x sub-tiles, so those go first; everything else follows in
        # consumption order to keep the PE from stalling at kernel start.
        # The K projection consumes (w_k chunk, x chunk) pairs sequentially;
        # the chunked interleave lets its first matmuls start ~3us in.  Wq
        # head 0 is prioritized over cos/sin (the Q0 projection's LDWEIGHTS
        # is the next PE stall point; RoPE-K on DVE can wait a little).
        w_k = singles.tile([128, NK, DH], dt.bfloat16, tag="wk")
        x_t0 = xpool.tile([128, NK, QC], dt.bfloat16, tag="xq", name="x_t0")
        for g in range(4):
            nc.sync.dma_start(out=w_k[:, g * 8:(g + 1) * 8],
                              in_=wk[:, g * 8:(g + 1) * 8])
            nc.sync.dma_start(out=x_t0[:, g * 8:(g + 1) * 8],
                              in_=x[:, 0, g * 8:(g + 1) * 8])

        w_q = singles.tile([128, NK, HPC, DH], dt.bfloat16, tag="wq")
        nc.sync.dma_start(out=w_q[:, :, 0], in_=wq[:, :, 0])

        c_cos = singles.tile([DH, S], dt.bfloat16, tag="cos")
        nc.sync.dma_start(out=c_cos, in_=cosT[:])
        c_sin = singles.tile([DH, S], dt.bfloat16, tag="sin")
        nc.sync.dma_start(out=c_sin, in_=sinm[:])

        for h in range(1, HPC):
            for g2 in range(2):
                nc.sync.dma_start(out=w_q[:, g2 * 16:(g2 + 1) * 16, h],
                                  in_=wq[:, g2 * 16:(g2 + 1) * 16, h])
        w_v = singles.tile([128, NK, DH], dt.bfloat16, tag="wv")
        nc.sync.dma_start(out=w_v, in_=wv[:])

        c_m01 = singles.tile([128, 4, QC], dt.bfloat16, tag="m01")
        nc.sync.dma_start(out=c_m01, in_=m01[:])
        c_id = singles.tile([128, 128], dt.bfloat16, tag="ident")
        nc.sync.dma_start(out=c_id, in_=ident[:])
        c_oc = singles.tile([128, 1], dt.bfloat16, tag="ones_col")
        nc.sync.dma_start(out=c_oc, in_=ones_col[:])
        c_or = singles.tile([1, 128], dt.float32, tag="ones_row")
        nc.sync.dma_start(out=c_or, in_=ones_row[:])
        c_bias = singles.tile([128, 1], dt.float32, tag="ebias")
        nc.vector.memset(c_bias, EXP_BIAS)

        # Wo resident from the start (first consumed during quarter-1
        # attention by the interleaved out-projection of quarter 0).
        w_o = singles.tile([128, HPC, DIN], dt.bfloat16, tag="wo")
        nc.sync.dma_start(out=w_o, in_=wo[:])

        # ---- long-lived activations ----
        kt = qkv.tile([DH, S], dt.bfloat16, tag="kt")
        vn = qkv.tile([128, NKT, DH], dt.bfloat16, tag="vn")     # V natural [j, d] tiles
        ctxT = [qkv.tile([DH, S], dt.bfloat16, tag=f"ctx{h}", name=f"ctx{h}") for h in range(HPC)]

        def rope_from_psum(ps, dst_slice, s0):
            """dst = ps*cos + rot_half(ps)*sinm over s-columns [s0, s0+QC)."""
            t1 = tpool.tile([DH, QC], dt.float32, tag="t1", name="t1")
            nc.vector.tensor_mul(t1, ps, c_cos[:, s0:s0 + QC])
            t2 = tpool.tile([DH, QC], dt.float32, tag="t2", name="t2")
            nc.vector.tensor_mul(t2[0:64, :], ps[64:128, :], c_sin[0:64, s0:s0 + QC])
            nc.vector.tensor_mul(t2[64:128, :], ps[0:64, :], c_sin[64:128, s0:s0 + QC])
            nc.vector.tensor_add(dst_slice, t1, t2)

        def emit_oproj_group(sq, st, ocw, alt):
            """One 1024-wide out-projection group: out rows of st-tile,
            columns [ocw*1024, (ocw+1)*1024). 8 matmuls + drain + DMA."""
            pso = ps_sc.tile([128, 2, QC], dt.float32, tag="sc", name="pso")
            for i in range(2):
                oc0 = ocw * OCW + i * QC
                for h in range(HPC):
                    nc.tensor.matmul(pso[:, i],
                                     lhsT=ctxT[h][:, st * 128:(st + 1) * 128],
                                     rhs=w_o[:, h, oc0:oc0 + QC],
                                     start=(h == 0), stop=(h == HPC - 1))
            if len(pending) == 0:
                # final group: split the drain in two so the end-of-kernel
                # copy+DMA tail is shorter
                for j2 in range(2):
                    sth = obuf.tile([128, QC], dt.bfloat16, tag="sth", name="sth")
                    if j2 == 0:
                        nc.scalar.copy(sth, pso[:, j2])
                    else:
                        nc.vector.tensor_copy(sth, pso[:, j2])
                    nc.sync.dma_start(
                        out=out[st * 128:(st + 1) * 128,
                                ocw * OCW + j2 * QC:ocw * OCW + (j2 + 1) * QC],
                        in_=sth)
                return
            stage = obuf.tile([128, OCW], dt.bfloat16, tag="stage", name="stage")
            if alt % 2 == 0:
                nc.scalar.copy(stage, pso)
            else:
                nc.vector.tensor_copy(stage, pso)
            nc.sync.dma_start(
                out=out[st * 128:(st + 1) * 128, ocw * OCW:(ocw + 1) * OCW],
                in_=stage)

        pending = []   # deferred out-proj groups of the previous quarter
        alt_ctr = [0]

        def pop_fillers(k):
            for _ in range(min(k, len(pending))):
                sq_, st_, ocw_ = pending.pop(0)
                emit_oproj_group(sq_, st_, ocw_, alt_ctr[0])
                alt_ctr[0] += 1

        for sq in range(NQ):
            s0 = sq * QC
            if sq == 0:
                x_t = x_t0
            else:
                x_t = xpool.tile([128, NK, QC], dt.bfloat16, tag="xq", name="x_t")
                for g in range(4):
                    nc.sync.dma_start(out=x_t[:, g * 8:(g + 1) * 8],
                                      in_=x[:, sq, g * 8:(g + 1) * 8])

            # K projection + RoPE
            psk = ps_acc.tile([DH, QC], dt.float32, tag="acc", name="psk")
            for k in range(NK):
                nc.tensor.matmul(psk, lhsT=w_k[:, k], rhs=x_t[:, k],
                                 start=(k == 0), stop=(k == NK - 1))
            rope_from_psum(psk, kt[:, s0:s0 + QC], s0)

            # Q projections + RoPE (per-quarter ring; consumed by this
            # quarter's attention only)
            qt = []
            for h in range(HPC):
                psq = ps_acc.tile([DH, QC], dt.float32, tag="acc", name="psq")
                for k in range(NK):
                    nc.tensor.matmul(psq, lhsT=w_q[:, k, h], rhs=x_t[:, k],
                                     start=(k == 0), stop=(k == NK - 1))
                qth = qpool.tile([DH, QC], dt.bfloat16, tag=f"qt{h}", name=f"qt{h}")
                rope_from_psum(psq, qth, s0)
                qt.append(qth)

            # V projection (transposed layout), then PE-transpose to natural
            psv = ps_acc.tile([DH, QC], dt.float32, tag="acc", name="psv")
            for k in range(NK):
                nc.tensor.matmul(psv, lhsT=w_v[:, k], rhs=x_t[:, k],
                                 start=(k == 0), stop=(k == NK - 1))
            vtmp = tpool.tile([DH, QC], dt.bfloat16, tag="vtmp", name="vtmp", bufs=1)
            nc.scalar.copy(vtmp, psv)
            for i in range(QC // 128):
                pvt = ps_sm.tile([128, 128], dt.bfloat16, tag="sm", name="pvt")
                nc.tensor.transpose(pvt, vtmp[:, i * 128:(i + 1) * 128], c_id)
                nc.scalar.copy(vn[:, sq * 4 + i], pvt)

            # ---- attention for this quarter's queries (causal) ----
            # Key tiles in PAIRS: scores for a pair land in one [128, 2, QC]
            # PSUM tile (2 banks) so a single wide exp covers both; the PE
            # runs one pair ahead of the PV matmuls to hide exp latency, and
            # the previous quarter's out-proj groups fill the remaining PE
            # idle (the attention phase is ACT-bound on exp).
            npairs = 2 * (sq + 1)
            njt = 2 * npairs
            for h in range(HPC):
                po = ps_acc.tile([DH, QC], dt.float32, tag="acc", name="po")
                swide = spool.tile([128, 2, QC], dt.bfloat16, tag="sacc", name="swide")

                def emit_scores(pr):
                    psc = ps_sc.tile([128, 2, QC], dt.float32, tag="sc", name="psc")
                    for half in range(2):
                        jt = 2 * pr + half
                        nc.tensor.matmul(psc[:, half],
                                         lhsT=kt[:, jt * KT:(jt + 1) * KT],
                                         rhs=qt[h], start=True, stop=True)
                    e = epool.tile([128, 2, QC], dt.bfloat16, tag="e", name="e")
                    nc.scalar.activation(out=e, in_=psc,
                                         func=mybir.ActivationFunctionType.Exp,
                                         bias=c_bias, scale=SCALE)
                    r = pr - (npairs - 2)
                    if r >= 0:
                        nc.vector.tensor_mul(e, e, c_m01[:, 2 * r:2 * r + 2])
                    return e

                def emit_pv(pr, e):
                    for half in range(2):
                        jt = 2 * pr + half
                        nc.tensor.matmul(po, lhsT=vn[:, jt], rhs=e[:, half],
                                         start=(jt == 0), stop=(jt == njt - 1))

                e_prev = emit_scores(0)
                for pr in range(1, npairs):
                    e_cur = emit_scores(pr)
                    emit_pv(pr - 1, e_prev)
                    if pr == 1:
                        nc.vector.tensor_add(swide, e_prev, e_cur)
                    else:
                        nc.vector.tensor_add(swide, swide, e_cur)
                    e_prev = e_cur
                emit_pv(npairs - 1, e_prev)

                # fold pair slots; PE filler (previous quarter's out-proj)
                # runs while the DVE/ACT softmax bookkeeping drains
                sfin = spool.tile([128, QC], dt.bfloat16, tag="sfin", name="sfin")
                nc.vector.tensor_add(sfin, swide[:, 0], swide[:, 1])
                pop_fillers(4)

                # normalization: ctx = O * (1 / colsum(E)) broadcast over d
                pcs = ps_sm.tile([1, QC], dt.float32, tag="sm", name="pcs")
                nc.tensor.matmul(pcs, lhsT=c_oc, rhs=sfin, start=True, stop=True)
                rec = tpool.tile([1, QC], dt.float32, tag="rec", name="rec", bufs=1)
                nc.vector.reciprocal_approx_fast(rec, pcs)
                prb = ps_sm.tile([128, QC], dt.float32, tag="sm", name="prb")
                nc.tensor.matmul(prb, lhsT=c_or, rhs=rec, start=True, stop=True)
                rbs = tpool.tile([128, QC], dt.float32, tag="rbs", name="rbs")
                nc.vector.tensor_copy(rbs, prb)
                nc.vector.tensor_mul(ctxT[h][:, s0:s0 + QC], po, rbs)

            # queue this quarter's out-proj groups (needs all 4 heads' ctxT)
            for st in range(4 * sq, 4 * sq + 4):
                for ocw in range(NOC):
                    pending.append((sq, st, ocw))

        # flush the last quarter's out-projection
        pop_fillers(len(pending))
    nc.finalize()
    return nc


def make_in_maps(input_tensor, cos, sin, Wq, Wk, Wv, Wo):
    """Host-side sharding + layout preparation. Returns list of 8 dicts."""
    x2 = np.ascontiguousarray(input_tensor.reshape(S, DIN))
    # x_host[p, sq, k, sc] = x2[512*sq+sc, 128*k+p]
    xt = x2.T.astype(BF16)                      # [DIN, S]
    x_host = np.ascontiguousarray(
        xt.reshape(NK, 128, NQ, QC).transpose(1, 2, 0, 3))

    cosT = np.ascontiguousarray(cos.T.astype(BF16))
    sinm = sin.T.astype(np.float32).copy()
    sinm[0:64, :] *= -1.0
    sinm = np.ascontiguousarray(sinm.astype(BF16))

    p_idx = np.arange(128)[:, None, None]
    r_idx = np.arange(4)[None, :, None]
    f_idx = np.arange(QC)[None, None, :]
    m01 = ((128 * r_idx + p_idx) <= f_idx).astype(BF16)

    ident = np.eye(128, dtype=BF16)
    ones_col = np.ones((128, 1), dtype=BF16)
    ones_row = np.ones((1, 128), dtype=np.float32)

    common = dict(x=x_host, cosT=cosT, sinm=sinm, m01=m01, ident=ident,
                  ones_col=ones_col, ones_row=ones_row)

    in_maps = []
    for c in range(NCORES):
        wq_s = Wq[:, c * DPC:(c + 1) * DPC].astype(BF16)
        wq_host = np.ascontiguousarray(
            wq_s.reshape(NK, 128, HPC, DH).transpose(1, 0, 2, 3))
        wk_s = Wk[:, c * DH:(c + 1) * DH].astype(BF16)
        wk_host = np.ascontiguousarray(wk_s.reshape(NK, 128, DH).transpose(1, 0, 2))
        wv_s = Wv[:, c * DH:(c + 1) * DH].astype(BF16)
        wv_host = np.ascontiguousarray(wv_s.reshape(NK, 128, DH).transpose(1, 0, 2))
        wo_s = Wo[c * DPC:(c + 1) * DPC, :].astype(BF16)
        wo_host = np.ascontiguousarray(wo_s.reshape(HPC, 128, DIN).transpose(1, 0, 2))
        in_maps.append(dict(common, wq=wq_host, wk=wk_host, wv=wv_host, wo=wo_host))
    return in_maps


def _numpy_fallback(input_tensor, attention_mask, cos, sin, Wq, Wk, Wv, Wo):
    x = input_tensor.astype(np.float32)
    b, s, _ = x.shape
    q = (x @ Wq).reshape(b, s, H, DH).transpose(0, 2, 1, 3)
    k = (x @ Wk).reshape(b, s, KV, DH).transpose(0, 2, 1, 3)
    v = (x @ Wv).reshape(b, s, KV, DH).transpose(0, 2, 1, 3)

    def rope(t):
        t1, t2 = t[..., :64], t[..., 64:]
        rot = np.concatenate([-t2, t1], axis=-1)
        return t * cos[None, None] + rot * sin[None, None]

    q, k = rope(q), rope(k)
    k = np.repeat(k, G, axis=1)
    v = np.repeat(v, G, axis=1)
    sc = np.einsum('bhqd,bhkd->bhqk', q, k)
    sc = np.where(attention_mask, -np.inf, sc) / np.float32(np.sqrt(DH))
    sc = sc - sc.max(axis=-1, keepdims=True)
    w = np.exp(sc)
    w = w / w.sum(axis=-1, keepdims=True)
    ctx = np.einsum('bhqk,bhkd->bhqd', w, v)
    ctx = ctx.transpose(0, 2, 1, 3).reshape(b, s, H * DH)
    return (ctx @ Wo).astype(np.float32)


_NC_CACHE = {}


def kernel(input_tensor, attention_mask, cos, sin, Wq, Wk, Wv, Wo):
    mask = np.asarray(attention_mask).reshape(S, S)
    causal = np.array_equal(mask, np.triu(np.ones((S, S), bool), k=1))
    if not causal:
        return _numpy_fallback(np.asarray(input_tensor), np.asarray(attention_mask),
                               np.asarray(cos), np.asarray(sin),
                               np.asarray(Wq), np.asarray(Wk),
                               np.asarray(Wv), np.asarray(Wo))

    if "nc" not in _NC_CACHE:
        _NC_CACHE["nc"] = build_nc()
    nc = _NC_CACHE["nc"]

    in_maps = make_in_maps(np.asarray(input_tensor), np.asarray(cos),
                           np.asarray(sin), np.asarray(Wq), np.asarray(Wk),
                           np.asarray(Wv), np.asarray(Wo))
    res = run_bass_kernel_spmd(nc, in_maps, core_ids=list(range(NCORES)))
    acc = np.zeros((S, DIN), np.float32)
    for r in res.results:
        acc += np.asarray(r["out"], dtype=np.float32)
    return acc.reshape(1, S, DIN)


# revision 39
# speedup vs baseline: 1.0067x; 1.0000x over previous
"""GroupQueryAttention on 8 TRN2 NeuronCores.  (v3 snapshot: 445,905 ns)

Strategy: tensor-parallel over heads. H=32 query heads, KV=8 kv heads,
group size G=4 -> each core owns exactly 1 kv head and its 4 query heads.
Per core:
  - QKV projections from a replicated (pre-transposed, channels-major) input
  - RoPE on Q/K (rotate-half, done on DVE across partition halves)
  - attention with scores computed TRANSPOSED ([keys, q] layout) so the
    exp(scores) tiles feed the V-matmul directly as the moving operand
    (no P-transpose needed); softmax normalization is deferred: O = V.E,
    then ctx = O * (1/colsum(E)) broadcast via a rank-1 matmul
  - partial output ctx @ Wo_shard  (row-shard of Wo)
Host sums the 8 partial outputs (the "all-reduce" of the row-parallel Wo).

v3 perf structure:
  - score/exp tiles in key-tile PAIRS ([128, 2, 512] PSUM; wide ACT exp)
  - PE software pipeline: scores for pair p+1 issue before PV of pair p
  - reciprocal_approx_fast for the softmax denominator
  - output projection groups accumulated in the wide ps_sc pool; each
    quarter's out-proj groups are interleaved into the NEXT quarter's
    attention as PE filler work (the attention phase is otherwise
    exp/ACT-bound and the PE would idle)
  - x input single-buffered (its reload hides under attention+filler),
    qt reduced to a per-quarter ring, Wo resident from the start
  - PSUM->SBUF drains split across ACT and DVE; output DMA'd per group

Causal mask: the attention_mask input is verified on host to be the
upper-triangular causal mask; the device program exploits causality by
skipping fully-masked key-tiles and applying 0/1 mask tiles on the 4
diagonal-crossing key-tiles of each query chunk. If the mask is ever not
causal, a numpy fallback computes the exact reference on host.

Compute dtype: bf16 on the PE (f32 PSUM accumulation), f32 softmax
bookkeeping. Output partials returned as bf16, summed on host in f32.
"""

import sys

sys.path.insert(0, "/opt/trn_rl_repo")

from contextlib import ExitStack

import numpy as np
import ml_dtypes

import concourse.bass as bass
import concourse.bacc as bacc
import concourse.tile as tile
from concourse import mybir
from concourse.bass_utils import run_bass_kernel_spmd

BF16 = ml_dtypes.bfloat16

S = 2048          # sequence length
DIN = 4096        # model dim
H, KV, DH = 32, 8, 128
G = H // KV       # 4 query heads per kv head
NCORES = 8
HPC = H // NCORES     # 4 query heads per core
DPC = HPC * DH        # 512 = per-core q-projection width

NQ = 4            # s-quarters (chunks of 512 queries)
QC = S // NQ      # 512
KT = 128          # key tile (partition dim of transposed scores)
NKT = S // KT     # 16 key tiles
NK = DIN // 128   # 32 contraction tiles for projections
SCALE = 1.0 / float(np.sqrt(DH))
EXP_BIAS = -10.0  # constant shift inside exp; cancels in normalization
OCW = 1024        # out-projection moving width
NOC = DIN // OCW  # 4 wide column groups


def build_nc():
    """Build the per-core Bass program (same program on all 8 cores; the
    per-core weight shards arrive via in_maps)."""
    nc = bacc.Bacc()
    dt = mybir.dt

    # ---- DRAM parameters (host-prepared layouts; all DMA-contiguous) ----
    # x[p, sq, k, sc] = x_orig[512*sq + sc, 128*k + p]   (channels-major)
    x = nc.declare_dram_parameter("x", [128, NQ, NK, QC], dt.bfloat16, isOutput=False)
    # wq[p, k, m, d] = Wq_shard[128*k + p, 128*m + d]
    wq = nc.declare_dram_parameter("wq", [128, NK, HPC, DH], dt.bfloat16, isOutput=False)
    # wk[p, k, d] = Wk_shard[128*k + p, d]
    wk = nc.declare_dram_parameter("wk", [128, NK, DH], dt.bfloat16, isOutput=False)
    wv = nc.declare_dram_parameter("wv", [128, NK, DH], dt.bfloat16, isOutput=False)
    # wo[p, h, n] = Wo_shard[128*h + p, n]
    wo = nc.declare_dram_parameter("wo", [128, HPC, DIN], dt.bfloat16, isOutput=False)
    # cosT[d, s] = cos[s, d]; sinm[d, s] = -sin[s, d] for d<64 else +sin[s, d]
    cosT = nc.declare_dram_parameter("cosT", [DH, S], dt.bfloat16, isOutput=False)
    sinm = nc.declare_dram_parameter("sinm", [DH, S], dt.bfloat16, isOutput=False)
    # m01[p, r, f] = 0.0 where 128*r + p > f else 1.0  (diagonal-tile masks)
    m01 = nc.declare_dram_parameter("m01", [128, 4, QC], dt.bfloat16, isOutput=False)
    ident = nc.declare_dram_parameter("ident", [128, 128], dt.bfloat16, isOutput=False)
    ones_col = nc.declare_dram_parameter("ones_col", [128, 1], dt.bfloat16, isOutput=False)
    ones_row = nc.declare_dram_parameter("ones_row", [1, 128], dt.float32, isOutput=False)
    out = nc.declare_dram_parameter("out", [S, DIN], dt.bfloat16, isOutput=True)

    with tile.TileContext(nc) as tc, ExitStack() as ctx:
        singles = ctx.enter_context(tc.tile_pool(name="singles", bufs=1))
        xpool = ctx.enter_context(tc.tile_pool(name="xpool", bufs=1))
        qpool = ctx.enter_context(tc.tile_pool(name="qpool", bufs=2))
        qkv = ctx.enter_context(tc.tile_pool(name="qkv", bufs=1))
        epool = ctx.enter_context(tc.tile_pool(name="epool", bufs=4))
        spool = ctx.enter_context(tc.tile_pool(name="spool", bufs=3))
        tpool = ctx.enter_context(tc.tile_pool(name="tpool", bufs=2))
        obuf = ctx.enter_context(tc.tile_pool(name="obuf", bufs=6))
        ps_acc = ctx.enter_context(tc.tile_pool(name="ps_acc", bufs=2, space="PSUM"))
        ps_sc = ctx.enter_context(tc.tile_pool(name="ps_sc", bufs=2, space="PSUM"))
        ps_sm = ctx.enter_context(tc.tile_pool(name="ps_sm", bufs=2, space="PSUM"))

        # ---- constants / weights resident in SBUF ----
        # DMA emission order matters: the K-projection consumes w_k and the
        # first x sub-tiles, so those go first; everything else follows in
        # consumption order to keep the PE from stalling at kernel start.
        # The K projection consumes (w_k chunk, x chunk) pairs sequentially;
        # the chunked interleave lets its first matmuls start ~3us in.  Wq
        # head 0 is prioritized over cos/sin (the Q0 projection's LDWEIGHTS
        # is the next PE stall point; RoPE-K on DVE can wait a little).
        w_k = singles.tile([128, NK, DH], dt.bfloat16, tag="wk")
        x_t0 = xpool.tile([128, NK, QC], dt.bfloat16, tag="xq", name="x_t0")
        for g in range(8):
            nc.sync.dma_start(out=w_k[:, g * 4:(g + 1) * 4],
                              in_=wk[:, g * 4:(g + 1) * 4])
            nc.sync.dma_start(out=x_t0[:, g * 4:(g + 1) * 4],
                              in_=x[:, 0, g * 4:(g + 1) * 4])

        w_q = singles.tile([128, NK, HPC, DH], dt.bfloat16, tag="wq")
        nc.sync.dma_start(out=w_q[:, :, 0], in_=wq[:, :, 0])

        c_cos = singles.tile([DH, S], dt.bfloat16, tag="cos")
        nc.sync.dma_start(out=c_cos, in_=cosT[:])
        c_sin = singles.tile([DH, S], dt.bfloat16, tag="sin")
        nc.sync.dma_start(out=c_sin, in_=sinm[:])

        for h in range(1, HPC):
            for g2 in range(2):
                nc.sync.dma_start(out=w_q[:, g2 * 16:(g2 + 1) * 16, h],
                                  in_=wq[:, g2 * 16:(g2 + 1) * 16, h])
        w_v = singles.tile([128, NK, DH], dt.bfloat16, tag="wv")
        nc.sync.dma_start(out=w_v, in_=wv[:])

        c_m01 = singles.tile([128, 4, QC], dt.bfloat16, tag="m01")
        nc.sync.dma_start(out=c_m01, in_=m01[:])
        c_id = singles.tile([128, 128], dt.bfloat16, tag="ident")
        nc.sync.dma_start(out=c_id, in_=ident[:])
        c_oc = singles.tile([128, 1], dt.bfloat16, tag="ones_col")
        nc.sync.dma_start(out=c_oc, in_=ones_col[:])
        c_or = singles.tile([1, 128], dt.float32, tag="ones_row")
        nc.sync.dma_start(out=c_or, in_=ones_row[:])
        c_bias = singles.tile([128, 1], dt.float32, tag="ebias")
        nc.vector.memset(c_bias, EXP_BIAS)

        # Wo resident from the start (first consumed during quarter-1
        # attention by the interleaved out-projection of quarter 0).
        w_o = singles.tile([128, HPC, DIN], dt.bfloat16, tag="wo")
        nc.sync.dma_start(out=w_o, in_=wo[:])

        # ---- long-lived activations ----
        kt = qkv.tile([DH, S], dt.bfloat16, tag="kt")
        vn = qkv.tile([128, NKT, DH], dt.bfloat16, tag="vn")     # V natural [j, d] tiles
        ctxT = [qkv.tile([DH, S], dt.bfloat16, tag=f"ctx{h}", name=f"ctx{h}") for h in range(HPC)]

        def rope_from_psum(ps, dst_slice, s0):
            """dst = ps*cos + rot_half(ps)*sinm over s-columns [s0, s0+QC)."""
            t1 = tpool.tile([DH, QC], dt.float32, tag="t1", name="t1")
            nc.vector.tensor_mul(t1, ps, c_cos[:, s0:s0 + QC])
            t2 = tpool.tile([DH, QC], dt.float32, tag="t2", name="t2")
            nc.vector.tensor_mul(t2[0:64, :], ps[64:128, :], c_sin[0:64, s0:s0 + QC])
            nc.vector.tensor_mul(t2[64:128, :], ps[0:64, :], c_sin[64:128, s0:s0 + QC])
            nc.vector.tensor_add(dst_slice, t1, t2)

        def emit_oproj_group(sq, st, ocw, alt):
            """One 1024-wide out-projection group: out rows of st-tile,
            columns [ocw*1024, (ocw+1)*1024). 8 matmuls + drain + DMA."""
            pso = ps_sc.tile([128, 2, QC], dt.float32, tag="sc", name="pso")
            for i in range(2):
                oc0 = ocw * OCW + i * QC
                for h in range(HPC):
                    nc.tensor.matmul(pso[:, i],
                                     lhsT=ctxT[h][:, st * 128:(st + 1) * 128],
                                     rhs=w_o[:, h, oc0:oc0 + QC],
                                     start=(h == 0), stop=(h == HPC - 1))
            if len(pending) == 0:
                # final group: split the drain in two so the end-of-kernel
                # copy+DMA tail is shorter
                for j2 in range(2):
                    sth = obuf.tile([128, QC], dt.bfloat16, tag="sth", name="sth")
                    if j2 == 0:
                        nc.scalar.copy(sth, pso[:, j2])
                    else:
                        nc.vector.tensor_copy(sth, pso[:, j2])
                    nc.sync.dma_start(
                        out=out[st * 128:(st + 1) * 128,
                                ocw * OCW + j2 * QC:ocw * OCW + (j2 + 1) * QC],
                        in_=sth)
                return
            stage = obuf.tile([128, OCW], dt.bfloat16, tag="stage", name="stage")
            if alt % 2 == 0:
                nc.scalar.copy(stage, pso)
            else:
                nc.vector.tensor_copy(stage, pso)
            nc.sync.dma_start(
                out=out[st * 128:(st + 1) * 128, ocw * OCW:(ocw + 1) * OCW],
                in_=stage)

        pending = []   # deferred out-proj groups of the previous quarter
        alt_ctr = [0]

        def pop_fillers(k):
            for _ in range(min(k, len(pending))):
                sq_, st_, ocw_ = pending.pop(0)
                emit_oproj_group(sq_, st_, ocw_, alt_ctr[0])
                alt_ctr[0] += 1

        for sq in range(NQ):
            s0 = sq * QC
            if sq == 0:
                x_t = x_t0
            else:
                x_t = xpool.tile([128, NK, QC], dt.bfloat16, tag="xq", name="x_t")
                for g in range(4):
                    nc.sync.dma_start(out=x_t[:, g * 8:(g + 1) * 8],
                                      in_=x[:, sq, g * 8:(g + 1) * 8])

            # K projection + RoPE
            psk = ps_acc.tile([DH, QC], dt.float32, tag="acc", name="psk")
            for k in range(NK):
                nc.tensor.matmul(psk, lhsT=w_k[:, k], rhs=x_t[:, k],
                                 start=(k == 0), stop=(k == NK - 1))
            rope_from_psum(psk, kt[:, s0:s0 + QC], s0)

            # Q projections + RoPE (per-quarter ring; consumed by this
            # quarter's attention only)
            qt = []
            for h in range(HPC):
                psq = ps_acc.tile([DH, QC], dt.float32, tag="acc", name="psq")
                for k in range(NK):
                    nc.tensor.matmul(psq, lhsT=w_q[:, k, h], rhs=x_t[:, k],
                                     start=(k == 0), stop=(k == NK - 1))
                qth = qpool.tile([DH, QC], dt.bfloat16, tag=f"qt{h}", name=f"qt{h}")
                rope_from_psum(psq, qth, s0)
                qt.append(qth)

            # V projection (transposed layout), then PE-transpose to natural
            psv = ps_acc.tile([DH, QC], dt.float32, tag="acc", name="psv")
            for k in range(NK):
                nc.tensor.matmul(psv, lhsT=w_v[:, k], rhs=x_t[:, k],
                                 start=(k == 0), stop=(k == NK - 1))
            vtmp = tpool.tile([DH, QC], dt.bfloat16, tag="vtmp", name="vtmp", bufs=1)
            nc.scalar.copy(vtmp, psv)
            for i in range(QC // 128):
                pvt = ps_sm.tile([128, 128], dt.bfloat16, tag="sm", name="pvt")
                nc.tensor.transpose(pvt, vtmp[:, i * 128:(i + 1) * 128], c_id)
                nc.scalar.copy(vn[:, sq * 4 + i], pvt)

            # ---- attention for this quarter's queries (causal) ----
            # Key tiles in PAIRS: scores for a pair land in one [128, 2, QC]
            # PSUM tile (2 banks) so a single wide exp covers both; the PE
            # runs one pair ahead of the PV matmuls to hide exp latency, and
            # the previous quarter's out-proj groups fill the remaining PE
            # idle (the attention phase is ACT-bound on exp).
            npairs = 2 * (sq + 1)
            njt = 2 * npairs
            for h in range(HPC):
                po = ps_acc.tile([DH, QC], dt.float32, tag="acc", name="po")
                swide = spool.tile([128, 2, QC], dt.bfloat16, tag="sacc", name="swide")

                def emit_scores(pr):
                    psc = ps_sc.tile([128, 2, QC], dt.float32, tag="sc", name="psc")
                    for half in range(2):
                        jt = 2 * pr + half
                        nc.tensor.matmul(psc[:, half],
                                         lhsT=kt[:, jt * KT:(jt + 1) * KT],
                                         rhs=qt[h], start=True, stop=True)
                    e = epool.tile([128, 2, QC], dt.bfloat16, tag="e", name="e")
                    nc.scalar.activation(out=e, in_=psc,
                                         func=mybir.ActivationFunctionType.Exp,
                                         bias=c_bias, scale=SCALE)
                    r = pr - (npairs - 2)
                    if r >= 0:
                        nc.vector.tensor_mul(e, e, c_m01[:, 2 * r:2 * r + 2])
                    return e

                def emit_pv(pr, e):
                    for half in range(2):
                        jt = 2 * pr + half
                        nc.tensor.matmul(po, lhsT=vn[:, jt], rhs=e[:, half],
                                         start=(jt == 0), stop=(jt == njt - 1))

                e_prev = emit_scores(0)
                for pr in range(1, npairs):
                    e_cur = emit_scores(pr)
                    emit_pv(pr - 1, e_prev)
                    if pr == 1:
                        nc.vector.tensor_add(swide, e_prev, e_cur)
                    else:
                        nc.vector.tensor_add(swide, swide, e_cur)
                    e_prev = e_cur
                emit_pv(npairs - 1, e_prev)

                # fold pair slots; PE filler (previous quarter's out-proj)
                # runs while the DVE/ACT softmax bookkeeping drains
                sfin = spool.tile([128, QC], dt.bfloat16, tag="sfin", name="sfin")
                nc.vector.tensor_add(sfin, swide[:, 0], swide[:, 1])
                pop_fillers(4)

                # normalization: ctx = O * (1 / colsum(E)) broadcast over d
                pcs = ps_sm.tile([1, QC], dt.float32, tag="sm", name="pcs")
                nc.tensor.matmul(pcs, lhsT=c_oc, rhs=sfin, start=True, stop=True)
                rec = tpool.tile([1, QC], dt.float32, tag="rec", name="rec", bufs=1)
                nc.vector.reciprocal_approx_fast(rec, pcs)
                prb = ps_sm.tile([128, QC], dt.float32, tag="sm", name="prb")
                nc.tensor.matmul(prb, lhsT=c_or, rhs=rec, start=True, stop=True)
                rbs = tpool.tile([128, QC], dt.float32, tag="rbs", name="rbs")
                nc.vector.tensor_copy(rbs, prb)
                nc.vector.tensor_mul(ctxT[h][:, s0:s0 + QC], po, rbs)

            # queue this quarter's out-proj groups (needs all 4 heads' ctxT)
            for st in range(4 * sq, 4 * sq + 4):
                for ocw in range(NOC):
                    pending.append((sq, st, ocw))

        # flush the last quarter's out-projection
        pop_fillers(len(pending))
    nc.finalize()
    return nc


def make_in_maps(input_tensor, cos, sin, Wq, Wk, Wv, Wo):
    """Host-side sharding + layout preparation. Returns list of 8 dicts."""
    x2 = np.ascontiguousarray(input_tensor.reshape(S, DIN))
    # x_host[p, sq, k, sc] = x2[512*sq+sc, 128*k+p]
    xt = x2.T.astype(BF16)                      # [DIN, S]
    x_host = np.ascontiguousarray(
        xt.reshape(NK, 128, NQ, QC).transpose(1, 2, 0, 3))

    cosT = np.ascontiguousarray(cos.T.astype(BF16))
    sinm = sin.T.astype(np.float32).copy()
    sinm[0:64, :] *= -1.0
    sinm = np.ascontiguousarray(sinm.astype(BF16))

    p_idx = np.arange(128)[:, None, None]
    r_idx = np.arange(4)[None, :, None]
    f_idx = np.arange(QC)[None, None, :]
    m01 = ((128 * r_idx + p_idx) <= f_idx).astype(BF16)

    ident = np.eye(128, dtype=BF16)
    ones_col = np.ones((128, 1), dtype=BF16)
    ones_row = np.ones((1, 128), dtype=np.float32)

    common = dict(x=x_host, cosT=cosT, sinm=sinm, m01=m01, ident=ident,
                  ones_col=ones_col, ones_row=ones_row)

    in_maps = []
    for c in range(NCORES):
        wq_s = Wq[:, c * DPC:(c + 1) * DPC].astype(BF16)
        wq_host = np.ascontiguousarray(
            wq_s.reshape(NK, 128, HPC, DH).transpose(1, 0, 2, 3))
        wk_s = Wk[:, c * DH:(c + 1) * DH].astype(BF16)
        wk_host = np.ascontiguousarray(wk_s.reshape(NK, 128, DH).transpose(1, 0, 2))
        wv_s = Wv[:, c * DH:(c + 1) * DH].astype(BF16)
        wv_host = np.ascontiguousarray(wv_s.reshape(NK, 128, DH).transpose(1, 0, 2))
        wo_s = Wo[c * DPC:(c + 1) * DPC, :].astype(BF16)
        wo_host = np.ascontiguousarray(wo_s.reshape(HPC, 128, DIN).transpose(1, 0, 2))
        in_maps.append(dict(common, wq=wq_host, wk=wk_host, wv=wv_host, wo=wo_host))
    return in_maps


def _numpy_fallback(input_tensor, attention_mask, cos, sin, Wq, Wk, Wv, Wo):
    x = input_tensor.astype(np.float32)
    b, s, _ = x.shape
    q = (x @ Wq).reshape(b, s, H, DH).transpose(0, 2, 1, 3)
    k = (x @ Wk).reshape(b, s, KV, DH).transpose(0, 2, 1, 3)
    v = (x @ Wv).reshape(b, s, KV, DH).transpose(0, 2, 1, 3)

    def rope(t):
        t1, t2 = t[..., :64], t[..., 64:]
        rot = np.concatenate([-t2, t1], axis=-1)
        return t * cos[None, None] + rot * sin[None, None]

    q, k = rope(q), rope(k)
    k = np.repeat(k, G, axis=1)
    v = np.repeat(v, G, axis=1)
    sc = np.einsum('bhqd,bhkd->bhqk', q, k)
    sc = np.where(attention_mask, -np.inf, sc) / np.float32(np.sqrt(DH))
    sc = sc - sc.max(axis=-1, keepdims=True)
    w = np.exp(sc)
    w = w / w.sum(axis=-1, keepdims=True)
    ctx = np.einsum('bhqk,bhkd->bhqd', w, v)
    ctx = ctx.transpose(0, 2, 1, 3).reshape(b, s, H * DH)
    return (ctx @ Wo).astype(np.float32)


_NC_CACHE = {}


def kernel(input_tensor, attention_mask, cos, sin, Wq, Wk, Wv, Wo):
    mask = np.asarray(attention_mask).reshape(S, S)
    causal = np.array_equal(mask, np.triu(np.ones((S, S), bool), k=1))
    if not causal:
        return _numpy_fallback(np.asarray(input_tensor), np.asarray(attention_mask),
                               np.asarray(cos), np.asarray(sin),
                               np.asarray(Wq), np.asarray(Wk),
                               np.asarray(Wv), np.asarray(Wo))

    if "nc" not in _NC_CACHE:
        _NC_CACHE["nc"] = build_nc()
    nc = _NC_CACHE["nc"]

    in_maps = make_in_maps(np.asarray(input_tensor), np.asarray(cos),
                           np.asarray(sin), np.asarray(Wq), np.asarray(Wk),
                           np.asarray(Wv), np.asarray(Wo))
    res = run_bass_kernel_spmd(nc, in_maps, core_ids=list(range(NCORES)))
    acc = np.zeros((S, DIN), np.float32)
    for r in res.results:
        acc += np.asarray(r["out"], dtype=np.float32)
    return acc.reshape(1, S, DIN)


# revision 40
# speedup vs baseline: 1.0408x; 1.0339x over previous
"""GroupQueryAttention on 8 TRN2 NeuronCores.  (v3 snapshot: 445,905 ns)

Strategy: tensor-parallel over heads. H=32 query heads, KV=8 kv heads,
group size G=4 -> each core owns exactly 1 kv head and its 4 query heads.
Per core:
  - QKV projections from a replicated (pre-transposed, channels-major) input
  - RoPE on Q/K (rotate-half, done on DVE across partition halves)
  - attention with scores computed TRANSPOSED ([keys, q] layout) so the
    exp(scores) tiles feed the V-matmul directly as the moving operand
    (no P-transpose needed); softmax normalization is deferred: O = V.E,
    then ctx = O * (1/colsum(E)) broadcast via a rank-1 matmul
  - partial output ctx @ Wo_shard  (row-shard of Wo)
Host sums the 8 partial outputs (the "all-reduce" of the row-parallel Wo).

v3 perf structure:
  - score/exp tiles in key-tile PAIRS ([128, 2, 512] PSUM; wide ACT exp)
  - PE software pipeline: scores for pair p+1 issue before PV of pair p
  - reciprocal_approx_fast for the softmax denominator
  - output projection groups accumulated in the wide ps_sc pool; each
    quarter's out-proj groups are interleaved into the NEXT quarter's
    attention as PE filler work (the attention phase is otherwise
    exp/ACT-bound and the PE would idle)
  - x input single-buffered (its reload hides under attention+filler),
    qt reduced to a per-quarter ring, Wo resident from the start
  - PSUM->SBUF drains split across ACT and DVE; output DMA'd per group

Causal mask: the attention_mask input is verified on host to be the
upper-triangular causal mask; the device program exploits causality by
skipping fully-masked key-tiles and applying 0/1 mask tiles on the 4
diagonal-crossing key-tiles of each query chunk. If the mask is ever not
causal, a numpy fallback computes the exact reference on host.

Compute dtype: bf16 on the PE (f32 PSUM accumulation), f32 softmax
bookkeeping. Output partials returned as bf16, summed on host in f32.
"""

import sys

sys.path.insert(0, "/opt/trn_rl_repo")

from contextlib import ExitStack

import numpy as np
import ml_dtypes

import concourse.bass as bass
import concourse.bacc as bacc
import concourse.tile as tile
from concourse import mybir
from concourse.bass_utils import run_bass_kernel_spmd

BF16 = ml_dtypes.bfloat16

S = 2048          # sequence length
DIN = 4096        # model dim
H, KV, DH = 32, 8, 128
G = H // KV       # 4 query heads per kv head
NCORES = 8
HPC = H // NCORES     # 4 query heads per core
DPC = HPC * DH        # 512 = per-core q-projection width

NQ = 4            # s-quarters (chunks of 512 queries)
QC = S // NQ      # 512
KT = 128          # key tile (partition dim of transposed scores)
NKT = S // KT     # 16 key tiles
NK = DIN // 128   # 32 contraction tiles for projections
SCALE = 1.0 / float(np.sqrt(DH))
EXP_BIAS = -10.0  # constant shift inside exp; cancels in normalization
OCW = 1024        # out-projection moving width
NOC = DIN // OCW  # 4 wide column groups


def build_nc():
    """Build the per-core Bass program (same program on all 8 cores; the
    per-core weight shards arrive via in_maps)."""
    nc = bacc.Bacc()
    dt = mybir.dt

    # ---- DRAM parameters (host-prepared layouts; all DMA-contiguous) ----
    # x[p, sq, k, sc] = x_orig[512*sq + sc, 128*k + p]   (channels-major)
    x = nc.declare_dram_parameter("x", [128, NQ, NK, QC], dt.bfloat16, isOutput=False)
    # wq[p, k, m, d] = Wq_shard[128*k + p, 128*m + d]
    wq = nc.declare_dram_parameter("wq", [128, NK, HPC, DH], dt.bfloat16, isOutput=False)
    # wk[p, k, d] = Wk_shard[128*k + p, d]
    wk = nc.declare_dram_parameter("wk", [128, NK, DH], dt.bfloat16, isOutput=False)
    wv = nc.declare_dram_parameter("wv", [128, NK, DH], dt.bfloat16, isOutput=False)
    # wo[p, h, n] = Wo_shard[128*h + p, n]
    wo = nc.declare_dram_parameter("wo", [128, HPC, DIN], dt.bfloat16, isOutput=False)
    # cosT[d, s] = cos[s, d]; sinm[d, s] = -sin[s, d] for d<64 else +sin[s, d]
    cosT = nc.declare_dram_parameter("cosT", [DH, S], dt.bfloat16, isOutput=False)
    sinm = nc.declare_dram_parameter("sinm", [DH, S], dt.bfloat16, isOutput=False)
    # m01[p, r, f] = 0.0 where 128*r + p > f else 1.0  (diagonal-tile masks)
    m01 = nc.declare_dram_parameter("m01", [128, 4, QC], dt.bfloat16, isOutput=False)
    ident = nc.declare_dram_parameter("ident", [128, 128], dt.bfloat16, isOutput=False)
    ones_col = nc.declare_dram_parameter("ones_col", [128, 1], dt.bfloat16, isOutput=False)
    ones_row = nc.declare_dram_parameter("ones_row", [1, 128], dt.float32, isOutput=False)
    out = nc.declare_dram_parameter("out", [S, DIN], dt.bfloat16, isOutput=True)

    with tile.TileContext(nc) as tc, ExitStack() as ctx:
        singles = ctx.enter_context(tc.tile_pool(name="singles", bufs=1))
        xpool = ctx.enter_context(tc.tile_pool(name="xpool", bufs=1))
        qpool = ctx.enter_context(tc.tile_pool(name="qpool", bufs=2))
        qkv = ctx.enter_context(tc.tile_pool(name="qkv", bufs=1))
        epool = ctx.enter_context(tc.tile_pool(name="epool", bufs=3))
        spool = ctx.enter_context(tc.tile_pool(name="spool", bufs=2))
        tpool = ctx.enter_context(tc.tile_pool(name="tpool", bufs=2))
        obuf = ctx.enter_context(tc.tile_pool(name="obuf", bufs=4))
        ps_acc = ctx.enter_context(tc.tile_pool(name="ps_acc", bufs=2, space="PSUM"))
        ps_sc = ctx.enter_context(tc.tile_pool(name="ps_sc", bufs=2, space="PSUM"))
        ps_sm = ctx.enter_context(tc.tile_pool(name="ps_sm", bufs=2, space="PSUM"))

        # ---- constants / weights resident in SBUF ----
        # DMA emission order matters: the K-projection consumes w_k and the
        # first x sub-tiles, so those go first; everything else follows in
        # consumption order to keep the PE from stalling at kernel start.
        # The K projection consumes (w_k chunk, x chunk) pairs sequentially;
        # the chunked interleave lets its first matmuls start ~3us in.  Wq
        # head 0 is prioritized over cos/sin (the Q0 projection's LDWEIGHTS
        # is the next PE stall point; RoPE-K on DVE can wait a little).
        w_k = singles.tile([128, NK, DH], dt.bfloat16, tag="wk")
        x_t0 = xpool.tile([128, NK, QC], dt.bfloat16, tag="xq", name="x_t0")
        for g in range(4):
            nc.sync.dma_start(out=w_k[:, g * 8:(g + 1) * 8],
                              in_=wk[:, g * 8:(g + 1) * 8])
            nc.sync.dma_start(out=x_t0[:, g * 8:(g + 1) * 8],
                              in_=x[:, 0, g * 8:(g + 1) * 8])

        w_q = singles.tile([128, NK, HPC, DH], dt.bfloat16, tag="wq")
        nc.sync.dma_start(out=w_q[:, :, 0], in_=wq[:, :, 0])

        c_cos = singles.tile([DH, S], dt.bfloat16, tag="cos")
        nc.sync.dma_start(out=c_cos, in_=cosT[:])
        c_sin = singles.tile([DH, S], dt.bfloat16, tag="sin")
        nc.sync.dma_start(out=c_sin, in_=sinm[:])

        for h in range(1, HPC):
            for g2 in range(2):
                nc.sync.dma_start(out=w_q[:, g2 * 16:(g2 + 1) * 16, h],
                                  in_=wq[:, g2 * 16:(g2 + 1) * 16, h])
        w_v = singles.tile([128, NK, DH], dt.bfloat16, tag="wv")
        nc.sync.dma_start(out=w_v, in_=wv[:])

        c_m01 = singles.tile([128, 4, QC], dt.bfloat16, tag="m01")
        nc.sync.dma_start(out=c_m01, in_=m01[:])
        c_id = singles.tile([128, 128], dt.bfloat16, tag="ident")
        nc.sync.dma_start(out=c_id, in_=ident[:])
        c_oc = singles.tile([128, 1], dt.bfloat16, tag="ones_col")
        nc.sync.dma_start(out=c_oc, in_=ones_col[:])
        c_or = singles.tile([1, 128], dt.float32, tag="ones_row")
        nc.sync.dma_start(out=c_or, in_=ones_row[:])
        c_bias = singles.tile([128, 1], dt.float32, tag="ebias")
        nc.vector.memset(c_bias, EXP_BIAS)

        # Wo resident from the start (first consumed during quarter-1
        # attention by the interleaved out-projection of quarter 0).
        w_o = singles.tile([128, HPC, DIN], dt.bfloat16, tag="wo")
        nc.sync.dma_start(out=w_o, in_=wo[:])

        # ---- long-lived activations ----
        kt = qkv.tile([DH, S], dt.bfloat16, tag="kt")
        vn = qkv.tile([128, NKT, DH], dt.bfloat16, tag="vn")     # V natural [j, d] tiles
        ctxT = [qkv.tile([DH, S], dt.bfloat16, tag=f"ctx{h}", name=f"ctx{h}") for h in range(HPC)]

        def rope_from_psum(ps, dst_slice, s0):
            """dst = ps*cos + rot_half(ps)*sinm over s-columns [s0, s0+QC)."""
            t1 = tpool.tile([DH, QC], dt.float32, tag="t1", name="t1")
            nc.vector.tensor_mul(t1, ps, c_cos[:, s0:s0 + QC])
            t2 = tpool.tile([DH, QC], dt.float32, tag="t2", name="t2")
            nc.vector.tensor_mul(t2[0:64, :], ps[64:128, :], c_sin[0:64, s0:s0 + QC])
            nc.vector.tensor_mul(t2[64:128, :], ps[0:64, :], c_sin[64:128, s0:s0 + QC])
            nc.vector.tensor_add(dst_slice, t1, t2)

        def emit_oproj_group(sq, st, ocw, alt):
            """One 1024-wide out-projection group: out rows of st-tile,
            columns [ocw*1024, (ocw+1)*1024). 8 matmuls + drain + DMA."""
            pso = ps_sc.tile([128, 2, QC], dt.float32, tag="sc", name="pso")
            for i in range(2):
                oc0 = ocw * OCW + i * QC
                for h in range(HPC):
                    nc.tensor.matmul(pso[:, i],
                                     lhsT=ctxT[h][:, st * 128:(st + 1) * 128],
                                     rhs=w_o[:, h, oc0:oc0 + QC],
                                     start=(h == 0), stop=(h == HPC - 1))
            if len(pending) == 0:
                # final group: split the drain in two so the end-of-kernel
                # copy+DMA tail is shorter
                for j2 in range(2):
                    sth = obuf.tile([128, QC], dt.bfloat16, tag="sth", name="sth")
                    if j2 == 0:
                        nc.scalar.copy(sth, pso[:, j2])
                    else:
                        nc.vector.tensor_copy(sth, pso[:, j2])
                    nc.sync.dma_start(
                        out=out[st * 128:(st + 1) * 128,
                                ocw * OCW + j2 * QC:ocw * OCW + (j2 + 1) * QC],
                        in_=sth)
                return
            stage = obuf.tile([128, OCW], dt.bfloat16, tag="stage", name="stage")
            if alt % 2 == 0:
                nc.scalar.copy(stage, pso)
            else:
                nc.vector.tensor_copy(stage, pso)
            nc.sync.dma_start(
                out=out[st * 128:(st + 1) * 128, ocw * OCW:(ocw + 1) * OCW],
                in_=stage)

        pending = []   # deferred out-proj groups of the previous quarter
        alt_ctr = [0]

        def pop_fillers(k):
            for _ in range(min(k, len(pending))):
                sq_, st_, ocw_ = pending.pop(0)
                emit_oproj_group(sq_, st_, ocw_, alt_ctr[0])
                alt_ctr[0] += 1

        for sq in range(NQ):
            s0 = sq * QC
            if sq == 0:
                x_t = x_t0
            else:
                x_t = xpool.tile([128, NK, QC], dt.bfloat16, tag="xq", name="x_t")
                for g in range(4):
                    nc.sync.dma_start(out=x_t[:, g * 8:(g + 1) * 8],
                                      in_=x[:, sq, g * 8:(g + 1) * 8])

            # K projection + RoPE
            psk = ps_acc.tile([DH, QC], dt.float32, tag="acc", name="psk")
            for k in range(NK):
                nc.tensor.matmul(psk, lhsT=w_k[:, k], rhs=x_t[:, k],
                                 start=(k == 0), stop=(k == NK - 1))
            rope_from_psum(psk, kt[:, s0:s0 + QC], s0)

            # Q projections + RoPE (per-quarter ring; consumed by this
            # quarter's attention only)
            qt = []
            for h in range(HPC):
                psq = ps_acc.tile([DH, QC], dt.float32, tag="acc", name="psq")
                for k in range(NK):
                    nc.tensor.matmul(psq, lhsT=w_q[:, k, h], rhs=x_t[:, k],
                                     start=(k == 0), stop=(k == NK - 1))
                qth = qpool.tile([DH, QC], dt.bfloat16, tag=f"qt{h}", name=f"qt{h}")
                rope_from_psum(psq, qth, s0)
                qt.append(qth)

            # V projection (transposed layout), then PE-transpose to natural
            psv = ps_acc.tile([DH, QC], dt.float32, tag="acc", name="psv")
            for k in range(NK):
                nc.tensor.matmul(psv, lhsT=w_v[:, k], rhs=x_t[:, k],
                                 start=(k == 0), stop=(k == NK - 1))
            vtmp = tpool.tile([DH, QC], dt.bfloat16, tag="vtmp", name="vtmp", bufs=1)
            nc.scalar.copy(vtmp, psv)
            for i in range(QC // 128):
                pvt = ps_sm.tile([128, 128], dt.bfloat16, tag="sm", name="pvt")
                nc.tensor.transpose(pvt, vtmp[:, i * 128:(i + 1) * 128], c_id)
                nc.scalar.copy(vn[:, sq * 4 + i], pvt)

            # ---- attention for this quarter's queries (causal) ----
            # Key tiles in PAIRS: scores for a pair land in one [128, 2, QC]
            # PSUM tile (2 banks) so a single wide exp covers both; the PE
            # runs one pair ahead of the PV matmuls to hide exp latency, and
            # the previous quarter's out-proj groups fill the remaining PE
            # idle (the attention phase is ACT-bound on exp).
            npairs = 2 * (sq + 1)
            njt = 2 * npairs
            for h in range(HPC):
                po = ps_acc.tile([DH, QC], dt.float32, tag="acc", name="po")
                swide = spool.tile([128, 2, QC], dt.bfloat16, tag="sacc", name="swide")

                def emit_scores(pr):
                    psc = ps_sc.tile([128, 2, QC], dt.float32, tag="sc", name="psc")
                    for half in range(2):
                        jt = 2 * pr + half
                        nc.tensor.matmul(psc[:, half],
                                         lhsT=kt[:, jt * KT:(jt + 1) * KT],
                                         rhs=qt[h], start=True, stop=True)
                    e = epool.tile([128, 2, QC], dt.bfloat16, tag="e", name="e")
                    nc.scalar.activation(out=e, in_=psc,
                                         func=mybir.ActivationFunctionType.Exp,
                                         bias=c_bias, scale=SCALE)
                    r = pr - (npairs - 2)
                    if r >= 0:
                        nc.vector.tensor_mul(e, e, c_m01[:, 2 * r:2 * r + 2])
                    return e

                def emit_pv(pr, e):
                    for half in range(2):
                        jt = 2 * pr + half
                        nc.tensor.matmul(po, lhsT=vn[:, jt], rhs=e[:, half],
                                         start=(jt == 0), stop=(jt == njt - 1))

                e_prev = emit_scores(0)
                for pr in range(1, npairs):
                    e_cur = emit_scores(pr)
                    emit_pv(pr - 1, e_prev)
                    if pr == 1:
                        nc.vector.tensor_add(swide, e_prev, e_cur)
                    else:
                        nc.vector.tensor_add(swide, swide, e_cur)
                    e_prev = e_cur
                emit_pv(npairs - 1, e_prev)

                # fold pair slots; PE filler (previous quarter's out-proj)
                # runs while the DVE/ACT softmax bookkeeping drains
                sfin = spool.tile([128, QC], dt.bfloat16, tag="sfin", name="sfin")
                nc.vector.tensor_add(sfin, swide[:, 0], swide[:, 1])
                pop_fillers(4)

                # normalization: ctx = O * (1 / colsum(E)) broadcast over d
                pcs = ps_sm.tile([1, QC], dt.float32, tag="sm", name="pcs")
                nc.tensor.matmul(pcs, lhsT=c_oc, rhs=sfin, start=True, stop=True)
                rec = tpool.tile([1, QC], dt.float32, tag="rec", name="rec", bufs=1)
                nc.vector.reciprocal_approx_fast(rec, pcs)
                prb = ps_sm.tile([128, QC], dt.float32, tag="sm", name="prb")
                nc.tensor.matmul(prb, lhsT=c_or, rhs=rec, start=True, stop=True)
                rbs = tpool.tile([128, QC], dt.float32, tag="rbs", name="rbs")
                nc.vector.tensor_copy(rbs, prb)
                nc.vector.tensor_mul(ctxT[h][:, s0:s0 + QC], po, rbs)

            # queue this quarter's out-proj groups (needs all 4 heads' ctxT)
            for st in range(4 * sq, 4 * sq + 4):
                for ocw in range(NOC):
                    pending.append((sq, st, ocw))

        # flush the last quarter's out-projection
        pop_fillers(len(pending))
    nc.finalize()
    return nc


def make_in_maps(input_tensor, cos, sin, Wq, Wk, Wv, Wo):
    """Host-side sharding + layout preparation. Returns list of 8 dicts."""
    x2 = np.ascontiguousarray(input_tensor.reshape(S, DIN))
    # x_host[p, sq, k, sc] = x2[512*sq+sc, 128*k+p]
    xt = x2.T.astype(BF16)                      # [DIN, S]
    x_host = np.ascontiguousarray(
        xt.reshape(NK, 128, NQ, QC).transpose(1, 2, 0, 3))

    cosT = np.ascontiguousarray(cos.T.astype(BF16))
    sinm = sin.T.astype(np.float32).copy()
    sinm[0:64, :] *= -1.0
    sinm = np.ascontiguousarray(sinm.astype(BF16))

    p_idx = np.arange(128)[:, None, None]
    r_idx = np.arange(4)[None, :, None]
    f_idx = np.arange(QC)[None, None, :]
    m01 = ((128 * r_idx + p_idx) <= f_idx).astype(BF16)

    ident = np.eye(128, dtype=BF16)
    ones_col = np.ones((128, 1), dtype=BF16)
    ones_row = np.ones((1, 128), dtype=np.float32)

    common = dict(x=x_host, cosT=cosT, sinm=sinm, m01=m01, ident=ident,
                  ones_col=ones_col, ones_row=ones_row)

    in_maps = []
    for c in range(NCORES):
        wq_s = Wq[:, c * DPC:(c + 1) * DPC].astype(BF16)
        wq_host = np.ascontiguousarray(
            wq_s.reshape(NK, 128, HPC, DH).transpose(1, 0, 2, 3))
        wk_s = Wk[:, c * DH:(c + 1) * DH].astype(BF16)
        wk_host = np.ascontiguousarray(wk_s.reshape(NK, 128, DH).transpose(1, 0, 2))
        wv_s = Wv[:, c * DH:(c + 1) * DH].astype(BF16)
        wv_host = np.ascontiguousarray(wv_s.reshape(NK, 128, DH).transpose(1, 0, 2))
        wo_s = Wo[c * DPC:(c + 1) * DPC, :].astype(BF16)
        wo_host = np.ascontiguousarray(wo_s.reshape(HPC, 128, DIN).transpose(1, 0, 2))
        in_maps.append(dict(common, wq=wq_host, wk=wk_host, wv=wv_host, wo=wo_host))
    return in_maps


def _numpy_fallback(input_tensor, attention_mask, cos, sin, Wq, Wk, Wv, Wo):
    x = input_tensor.astype(np.float32)
    b, s, _ = x.shape
    q = (x @ Wq).reshape(b, s, H, DH).transpose(0, 2, 1, 3)
    k = (x @ Wk).reshape(b, s, KV, DH).transpose(0, 2, 1, 3)
    v = (x @ Wv).reshape(b, s, KV, DH).transpose(0, 2, 1, 3)

    def rope(t):
        t1, t2 = t[..., :64], t[..., 64:]
        rot = np.concatenate([-t2, t1], axis=-1)
        return t * cos[None, None] + rot * sin[None, None]

    q, k = rope(q), rope(k)
    k = np.repeat(k, G, axis=1)
    v = np.repeat(v, G, axis=1)
    sc = np.einsum('bhqd,bhkd->bhqk', q, k)
    sc = np.where(attention_mask, -np.inf, sc) / np.float32(np.sqrt(DH))
    sc = sc - sc.max(axis=-1, keepdims=True)
    w = np.exp(sc)
    w = w / w.sum(axis=-1, keepdims=True)
    ctx = np.einsum('bhqk,bhkd->bhqd', w, v)
    ctx = ctx.transpose(0, 2, 1, 3).reshape(b, s, H * DH)
    return (ctx @ Wo).astype(np.float32)


_NC_CACHE = {}


def kernel(input_tensor, attention_mask, cos, sin, Wq, Wk, Wv, Wo):
    mask = np.asarray(attention_mask).reshape(S, S)
    causal = np.array_equal(mask, np.triu(np.ones((S, S), bool), k=1))
    if not causal:
        return _numpy_fallback(np.asarray(input_tensor), np.asarray(attention_mask),
                               np.asarray(cos), np.asarray(sin),
                               np.asarray(Wq), np.asarray(Wk),
                               np.asarray(Wv), np.asarray(Wo))

    if "nc" not in _NC_CACHE:
        _NC_CACHE["nc"] = build_nc()
    nc = _NC_CACHE["nc"]

    in_maps = make_in_maps(np.asarray(input_tensor), np.asarray(cos),
                           np.asarray(sin), np.asarray(Wq), np.asarray(Wk),
                           np.asarray(Wv), np.asarray(Wo))
    res = run_bass_kernel_spmd(nc, in_maps, core_ids=list(range(NCORES)))
    acc = np.zeros((S, DIN), np.float32)
    for r in res.results:
        acc += np.asarray(r["out"], dtype=np.float32)
    return acc.reshape(1, S, DIN)
